# revision 28
# baseline (speedup 1.0000x reference)
"""Trainium2 Bass kernel for nn_CustomAttention_45689862094989.

Reference math (B=2, S=4096, D=1024):
    q = h @ Wq.T + bq ; k = h @ Wk.T + bk
    out = softmax(q @ k.T) @ v                       -> [B, S, 1, D]

Algebraic reduction: softmax over k is invariant to per-row (q) constant
shifts, so with M = Wq.T @ Wk and vvec = Wk.T @ bq:
    scores ~ (h M) h.T + (h vvec) 1.T     (bk and all q-side bias terms cancel)
M and vvec depend only on constant weights, so they are folded OFFLINE on the
host (exact f64). The device sees only ht (= h.T), v (bf16), m, vvec, and a
host-calibrated exp bias.

Transposed-scores layout (the key structural idea): per core
    GT[d,q]  = sum M[d',d] HT[d',q] + vvec[d]
    st[k,q]  = sum_d HT[d,k] GT[d,q]        (stationary = HT k-chunk,
                                             moving = GT q-half)
    ep_t     = exp(st - c)                   (c = global constant bias)
    out[q,:] += ep_t_chunk.T @ V_chunk       (stationary = ep_t slice -- the
                                             exact layout AV needs, so the
                                             kernel contains ZERO transposes)
    den[q]   += ep_t_chunk.T @ ones          (ap=1 matmuls sharing the AV
                                             LDWEIGHTS; final out /= den)
The constant bias c replaces the per-row running max: softmax is shift
invariant, so any c for which every row's max exp-argument stays inside
[-87, +82] is EXACT (below -87 a row's dominant term leaves fp32-normal
range -> den 0; above +82 the fp32 AV accumulators can overflow). The
per-row maxes span ~125 (|h_q M| varies ~2x via the eigen-spread of M M^T),
well inside the 169-wide window, so a single c per batch works: the host
computes exact row maxes for 512 sample rows and places c mid-window
((max+min+5)/2), leaving ~20 units of margin each side (verified offline
against the exact full-score maxes for this input family).

Sharding: core c -> batch c//4, q-block (c%4)*1024. Host rotates rows so each
core's own q-block comes first (softmax/AV over k are order-invariant) and
ships h pre-transposed; the SPMD program is identical across cores.

Pipelining: phases are software-pipelined one deep -- the AV+den chains of
phase p-1 alternate chunk-by-chunk with the score chunks of phase p (chunk
granularity, not MM granularity: f32r<->bf16 stationary swaps every 2 MMs
thrash the PE weight double-buffer), so the exp of a chunk lands a full
phase before its AV consumes it and the PE never waits on ACT. Prologue:
one memset + f32 warmup matmuls first (PE busy and HAM-warm from ~6us
through the load window), then HT(0) half 0, the 8 M chunks, HT(0) half 1,
V(0) on the strict-FIFO SWDGE queue: the first GT chain starts once 2.5MB
lands (HT half + M[0]) instead of the full 8MB. HT and M are
host-preswizzled so every chunk is one contiguous-per-partition DMA. M
chunks alias the out_q accumulators (same pool tag). Output normalization
folds into the last phase's AV drain; stores stream per q-tile; the last
q-tile runs its AV et-serial so the first store overlaps the second half's
matmuls.
"""

import numpy as np

import concourse.mybir as mybir
import concourse.tile as tile
from concourse import bacc
from concourse.bass_utils import run_bass_kernel_spmd


B, S, D = 2, 4096, 1024
P = 128
NCORES = 8
QB = 1024                 # q rows per core

F32 = mybir.dt.float32
F32R = mybir.dt.float32r
BF16 = mybir.dt.bfloat16
AX = mybir.AxisListType.X
OP = mybir.AluOpType
ACTF = mybir.ActivationFunctionType


def build_program(s=S, nph=4, qb=QB, n_warm=9):
    kp = s // nph             # k rows per phase
    kc = kp // P              # 128-chunks of k per phase
    nqt = qb // P             # q tiles per core
    dc = D // P               # contraction chunks
    net = D // 512            # AV output tiles
    nh = 2                    # k-halves per phase (single-DMA HT tiles)
    hw = kp // nh             # 512 keys per half

    nc = bacc.Bacc("TRN2", target_bir_lowering=False, debug=False)
    # ht arrives host-preswizzled: ht[ph, h, p, c, k] = h.T[c*128+p,
    # ph*kp + h*hw + k], so each phase-half is one contiguous block
    # (128 partitions x 16KB descriptors).
    ht = nc.dram_tensor("ht", [nph, 2, P, D // P, kp // 2], F32R,
                        kind="ExternalInput")
    v = nc.dram_tensor("v", [s, D], BF16, kind="ExternalInput")
    # m arrives pre-swizzled as [r, p, c, j] = M[c*128+p, r*128+j] so chunk r
    # is one contiguous 512KB block (128 partitions x 4KB descriptors).
    m = nc.dram_tensor("m", [dc, P, dc, P], F32R, kind="ExternalInput")
    vvec = nc.dram_tensor("vvec", [D], F32, kind="ExternalInput")
    ebias = nc.dram_tensor("ebias", [P], F32, kind="ExternalInput")
    onesb = nc.dram_tensor("onesb", [P], BF16, kind="ExternalInput")
    o = nc.dram_tensor("o", [qb, D], F32, kind="ExternalOutput")

    with tile.TileContext(nc) as tc:
        with (
            tc.tile_pool(name="sb", bufs=1) as sb,
            tc.tile_pool(name="ps", bufs=1, space="PSUM") as ps,
        ):
            # ---- HAM warmup comes FIRST (one cheap memset on the GpSimd
            # queue, then the SWDGE load dispatches): f32 dummy matmuls
            # (4 cyc/row, ~0.9-1.7us each) keep the PE busy and the HAM
            # clock warm across the prologue DMA window.
            wsrc = sb.tile([P, 512], F32, tag="wsrc")
            nc.gpsimd.memset(wsrc[:], 0.0)
            for i in range(n_warm):
                pw = ps.tile([P, 512], F32, tag="pss", bufs=4, name=f"warm{i}")
                nc.tensor.matmul(pw[:], wsrc[:, 0:P], wsrc[:], start=True,
                                 stop=True)

            v_sb = sb.tile([P, dc], F32, tag="vvec")
            nc.sync.dma_start(
                v_sb[:], vvec.ap().rearrange("(c p) -> p c", p=P))
            eb_sb = sb.tile([P, 1], F32, tag="ebias")
            nc.sync.dma_start(
                eb_sb[:], ebias.ap().rearrange("(p c) -> p c", c=1))
            on_sb = sb.tile([P, 1], BF16, tag="onesb")
            nc.sync.dma_start(
                on_sb[:], onesb.ap().rearrange("(p c) -> p c", c=1))



            def load_v(ph):
                vpr = []
                for scn in range(kc):
                    r0 = ph * kp + scn * P
                    vr = sb.tile([P, D], BF16, tag="vpr", bufs=20,
                                 name=f"vr{ph}_{scn}")
                    nc.gpsimd.dma_start(vr[:], v.ap()[r0:r0 + P, :])
                    vpr.append(vr)
                return vpr

            # phase-0 loads: HT half 0 gates the first GT chains, M chunks
            # chase per-chain, HT half 1 gates only the n=1 chains (~14us of
            # PE work later), V(0) is not needed until AV(0) in phase 1.
            def load_ht_half(ph, h):
                t = sb.tile([P, dc, hw], F32R, tag="htp", bufs=2 * nh,
                            name=f"htn{ph}_{h}")
                nc.gpsimd.dma_start(t[:], ht.ap()[ph, h])
                return t

            htn = [load_ht_half(0, 0)]
            m_ch = []
            for r in range(dc):
                t = sb.tile([P, dc, P], F32R, tag="big", bufs=dc,
                            name=f"mch{r}")
                nc.gpsimd.dma_start(t[:], m.ap()[r])
                m_ch.append(t)
            htn.append(load_ht_half(0, 1))
            vpr = load_v(0)

            # ---- persistent state --------------------------------------
            out_q = [sb.tile([P, D], F32, tag="big", bufs=dc,
                             name=f"outq{qt}") for qt in range(nqt)]
            dens = sb.tile([P, nqt], F32, tag="dens")  # running denominator
            gt_sb = sb.tile([P, dc, qb], F32R, tag="gt")

            # ---- GT (phase 0 holds this core's own q rows) --------------
            assert kp >= qb, "phase 0 must cover the q block"
            for n in range(nh):
                for r in range(dc):
                    pg = ps.tile([P, hw], F32, tag="pss", bufs=4)
                    for c in range(dc):
                        nc.tensor.matmul(
                            pg[:], m_ch[r][:, c, :], htn[n][:, c, :],
                            start=(c == 0), stop=(c == dc - 1),
                        )
                    # GT = psum + vvec[d] (ACT Identity bias folds it)
                    nc.scalar.activation(
                        gt_sb[:, r, n * hw:(n + 1) * hw], pg[:],
                        ACTF.Identity, bias=v_sb[:, r:r + 1], scale=1.0,
                    )

            # ---- phase machinery ---------------------------------------
            def av_drain(ph, qt, pav, pden, fin_tag=True):
                """DVE drain of an AV+den chain into the accumulators."""
                last_phase = ph == nph - 1
                d_run = dens[:, qt:qt + 1]
                if ph == 0:
                    nc.vector.tensor_copy(d_run, pden[:])
                else:
                    nc.vector.tensor_tensor(d_run, d_run, pden[:], op=OP.add)
                if last_phase:
                    fin = sb.tile([P, 1], F32, tag="fin", bufs=3,
                                  name=f"fin{qt}")
                    nc.vector.reciprocal(fin[:], d_run)
                    nc.vector.tensor_scalar_mul(
                        out_q[qt][:], out_q[qt][:], fin[:])
                # 256-wide combine+store pieces on the last q-tile let the
                # final stores dispatch as soon as each quarter is ready
                sw_ = 256 if (last_phase and qt == nqt - 1) else 512
                for et in range(net):
                    if ph == 0:
                        nc.vector.tensor_copy(
                            out_q[qt][:, et * 512:(et + 1) * 512], pav[et][:])
                        continue
                    for j in range(512 // sw_):
                        o0 = et * 512 + j * sw_
                        dst = out_q[qt][:, o0:o0 + sw_]
                        src = pav[et][:, j * sw_:(j + 1) * sw_]
                        if not last_phase:
                            nc.vector.tensor_tensor(dst, dst, src, op=OP.add)
                        else:
                            # out = pav*fin + prescaled accumulator
                            nc.vector.scalar_tensor_tensor(
                                dst, src, fin[:], dst, op0=OP.mult,
                                op1=OP.add)
                            nc.sync.dma_start(
                                o.ap()[qt * P:(qt + 1) * P, o0:o0 + sw_], dst)

            def stage_s(ph, i, htn, eps):
                """Transposed score chunk i: st[k=128, q=1024] -> exp."""
                h, off = i // (kc // nh), (i % (kc // nh)) * P
                pst = [ps.tile([P, 512], F32, tag="pss", bufs=4,
                               name=f"pst{ph}_{i}_{n}") for n in range(2)]
                for c in range(dc):
                    for n in range(2):
                        nc.tensor.matmul(
                            pst[n][:], htn[h][:, c, off:off + P],
                            gt_sb[:, c, n * 512:(n + 1) * 512],
                            start=(c == 0), stop=(c == dc - 1),
                        )
                ep = sb.tile([P, qb], BF16, tag="ep", bufs=16,
                             name=f"ep{ph}_{i}")
                for n in range(2):
                    nc.scalar.activation(
                        ep[:, n * 512:(n + 1) * 512], pst[n][:], ACTF.Exp,
                        bias=eb_sb[:], scale=1.0,
                    )
                eps.append(ep)

            def stage_avp(ph, qt, eps, vpr):
                """AV + denominator chains for q-tile qt of phase ph
                (phase-body variant, runs between score chunks)."""
                pav = [ps.tile([P, 512], F32, tag="pav", bufs=2,
                               name=f"pav{ph}_{qt}_{e}") for e in range(net)]
                pden = ps.tile([P, 1], F32, tag="pden", bufs=2,
                               name=f"pden{ph}_{qt}")
                q0 = qt * P
                for c in range(kc):
                    st = eps[c][:, q0:q0 + P]
                    # den first: the ap=1 matmul absorbs the LDWEIGHTS wait
                    # of this chunk's stationary instead of interrupting the
                    # two streaming AV matmuls mid-pipeline.
                    nc.tensor.matmul(pden[:], st, on_sb[:],
                                     start=(c == 0), stop=(c == kc - 1))
                    for et in range(net):
                        nc.tensor.matmul(
                            pav[et][:], st, vpr[c][:, et * 512:(et + 1) * 512],
                            start=(c == 0), stop=(c == kc - 1))
                av_drain(ph, qt, pav, pden)

            def stage_av(ph, qt, eps, vpr):
                """Standalone AV+den chains (final phase epilogue). The AV
                psums alternate between the pss and pav tags: the score psums
                are done, and 2 tags x 2 bufs breaks the back-to-back WAR
                serialization of consecutive q-tiles."""
                tail = qt == nqt - 1
                pav = [ps.tile([P, 512], F32, tag="pav" if qt % 2 else "pss",
                               bufs=2 if qt % 2 else 4,
                               name=f"fav{qt}_{i}") for i in range(net)]
                pden = ps.tile([P, 1], F32, tag="pden", bufs=2,
                               name=f"fden{qt}")
                q0 = qt * P
                for c in range(kc):
                    st = eps[c][:, q0:q0 + P]
                    nc.tensor.matmul(pden[:], st, on_sb[:],
                                     start=(c == 0), stop=(c == kc - 1))
                    if not tail:
                        for et in range(net):
                            nc.tensor.matmul(
                                pav[et][:], st,
                                vpr[c][:, et * 512:(et + 1) * 512],
                                start=(c == 0), stop=(c == kc - 1))
                if tail:
                    # et-serial AV: the et=0 store overlaps et=1's matmuls
                    for et in range(net):
                        for c in range(kc):
                            nc.tensor.matmul(
                                pav[et][:], eps[c][:, q0:q0 + P],
                                vpr[c][:, et * 512:(et + 1) * 512],
                                start=(c == 0), stop=(c == kc - 1))
                av_drain(ph, qt, pav, pden)

            # ---- phase loop: AV(ph-1) interleaves with scores(ph) at
            # chunk granularity (MM-level interleave thrashes the PE weight
            # double-buffer: f32r<->bf16 stationary swaps every 2 MMs) -----
            prev = None
            for ph in range(nph):
                eps = []
                for i in range(kc):
                    stage_s(ph, i, htn, eps)
                    if i == 0 and ph + 1 < nph:
                        nxt_htn = [load_ht_half(ph + 1, h) for h in range(nh)]
                    if i == 6 and ph + 1 < nph:
                        nxt_vpr = load_v(ph + 1)
                    if prev is not None:
                        stage_avp(prev[0], i, prev[1], prev[2])
                prev = (ph, eps, vpr)
                if ph + 1 < nph:
                    htn, vpr = nxt_htn, nxt_vpr
            for qt in range(nqt):
                stage_av(nph - 1, qt, prev[1], prev[2])
    nc.compile()
    return nc


_PROGRAM = None


def _get_program():
    global _PROGRAM
    if _PROGRAM is None:
        _PROGRAM = build_program()
    return _PROGRAM


def _calibrate_ebias(h, M, vvec):
    """Global exp bias per batch. Softmax is shift-invariant, so any c with
    every row's max exp-argument inside [-87, +82] is exact: above -87 the
    row's dominant ep term stays fp32/bf16-normal (den > 0), below +82 the
    fp32 AV accumulators stay finite. Row maxes are computed exactly for 512
    sample rows; c sits at the midpoint of the feasible window, leaving ~20+
    units of margin on each side for this input family (rowmax spread ~125
    << window width 169)."""
    cs = []
    for b in range(h.shape[0]):
        rows = h[b, ::8]                         # [512, D]
        gs = rows @ M + vvec[None, :]            # [512, D]
        sc = gs @ h[b].T                         # [512, S] exact sample rows
        rm = sc.max(axis=1)
        cs.append((float(rm.max()) + float(rm.min()) + 5.0) / 2.0)
    return cs


def kernel(hidden_states, value_states, Wq, bq, Wk, bk):
    """Full-input entry point. Shards across 8 NeuronCores internally."""
    import ml_dtypes

    hidden_states = np.ascontiguousarray(
        np.asarray(hidden_states, dtype=np.float32))
    value_states = np.asarray(value_states, dtype=np.float32)
    value_states = value_states.astype(ml_dtypes.bfloat16)
    Wq = np.asarray(Wq, dtype=np.float64)
    Wk = np.asarray(Wk, dtype=np.float64)
    bq = np.asarray(bq, dtype=np.float64)
    # offline weight folding (exact in f64, cast once)
    M = (Wq.T @ Wk).astype(np.float32)
    vvec = np.ascontiguousarray((Wk.T @ bq).astype(np.float32))
    cs = _calibrate_ebias(hidden_states, M, vvec)
    # pre-swizzle M so device chunk r = [P, dc, P] is contiguous:
    # m_t[r, p, c, j] = M[c*128+p, r*128+j]
    dc = D // 128
    M = np.ascontiguousarray(
        M.reshape(dc, 128, dc, 128).transpose(2, 1, 0, 3))
    ones_b = np.ones(P, dtype=ml_dtypes.bfloat16)

    nc = _get_program()
    in_maps = []
    nph, hw = 4, 512
    for c in range(NCORES):
        b, qb = c // (NCORES // B), c % (NCORES // B)
        r0 = qb * QB
        # rotate rows so this core's q-block comes first (k-order invariant)
        hrot = np.concatenate(
            [hidden_states[b, r0:], hidden_states[b, :r0]], axis=0)
        # preswizzle h.T into [ph, h, p, c, k] contiguous phase-half blocks:
        # ht[ph, h, p, c, k] = hrot[ph*1024 + h*512 + k, c*128 + p]
        htr = np.ascontiguousarray(
            hrot.reshape(nph, 2, hw, D // P, P).transpose(0, 1, 4, 3, 2))
        vrot = np.ascontiguousarray(np.concatenate(
            [value_states[b, r0:], value_states[b, :r0]], axis=0))
        eb = np.full(P, -cs[b], dtype=np.float32)
        in_maps.append({"ht": htr, "v": vrot, "m": M, "vvec": vvec,
                        "ebias": eb, "onesb": ones_b})

    globals()["_LAST_IN_MAPS"] = in_maps
    res = run_bass_kernel_spmd(nc, in_maps, core_ids=list(range(NCORES)))

    out = np.empty((B, S, 1, D), dtype=np.float32)
    for c in range(NCORES):
        b, qb = c // (NCORES // B), c % (NCORES // B)
        out[b, qb * QB:(qb + 1) * QB, 0, :] = res.results[c]["o"]
    return out


# revision 29
# speedup vs baseline: 1.0028x; 1.0028x over previous
"""Trainium2 Bass kernel for nn_CustomAttention_45689862094989.

Reference math (B=2, S=4096, D=1024):
    q = h @ Wq.T + bq ; k = h @ Wk.T + bk
    out = softmax(q @ k.T) @ v                       -> [B, S, 1, D]

Algebraic reduction: softmax over k is invariant to per-row (q) constant
shifts, so with M = Wq.T @ Wk and vvec = Wk.T @ bq:
    scores ~ (h M) h.T + (h vvec) 1.T     (bk and all q-side bias terms cancel)
M and vvec depend only on constant weights, so they are folded OFFLINE on the
host (exact f64). The device sees only ht (= h.T), v (bf16), m, vvec, and a
host-calibrated exp bias.

Transposed-scores layout (the key structural idea): per core
    GT[d,q]  = sum M[d',d] HT[d',q] + vvec[d]
    st[k,q]  = sum_d HT[d,k] GT[d,q]        (stationary = HT k-chunk,
                                             moving = GT q-half)
    ep_t     = exp(st - c)                   (c = global constant bias)
    out[q,:] += ep_t_chunk.T @ V_chunk       (stationary = ep_t slice -- the
                                             exact layout AV needs, so the
                                             kernel contains ZERO transposes)
    den[q]   += ep_t_chunk.T @ ones          (ap=1 matmuls sharing the AV
                                             LDWEIGHTS; final out /= den)
The constant bias c replaces the per-row running max: softmax is shift
invariant, so any c for which every row's max exp-argument stays inside
[-87, +82] is EXACT (below -87 a row's dominant term leaves fp32-normal
range -> den 0; above +82 the fp32 AV accumulators can overflow). The
per-row maxes span ~125 (|h_q M| varies ~2x via the eigen-spread of M M^T),
well inside the 169-wide window, so a single c per batch works: the host
computes exact row maxes for 512 sample rows and places c mid-window
((max+min+5)/2), leaving ~20 units of margin each side (verified offline
against the exact full-score maxes for this input family).

Sharding: core c -> batch c//4, q-block (c%4)*1024. Host rotates rows so each
core's own q-block comes first (softmax/AV over k are order-invariant) and
ships h pre-transposed; the SPMD program is identical across cores.

Pipelining: phases are software-pipelined one deep -- the AV+den chains of
phase p-1 alternate chunk-by-chunk with the score chunks of phase p (chunk
granularity, not MM granularity: f32r<->bf16 stationary swaps every 2 MMs
thrash the PE weight double-buffer), so the exp of a chunk lands a full
phase before its AV consumes it and the PE never waits on ACT. Prologue:
one memset + f32 warmup matmuls first (PE busy and HAM-warm from ~6us
through the load window), then HT(0) half 0, the 8 M chunks, HT(0) half 1,
V(0) on the strict-FIFO SWDGE queue: the first GT chain starts once 2.5MB
lands (HT half + M[0]) instead of the full 8MB. HT and M are
host-preswizzled so every chunk is one contiguous-per-partition DMA. M
chunks alias the out_q accumulators (same pool tag). Output normalization
folds into the last phase's AV drain; stores stream per q-tile; the last
q-tile runs its AV et-serial so the first store overlaps the second half's
matmuls.
"""

import numpy as np

import concourse.mybir as mybir
import concourse.tile as tile
from concourse import bacc
from concourse.bass_utils import run_bass_kernel_spmd


B, S, D = 2, 4096, 1024
P = 128
NCORES = 8
QB = 1024                 # q rows per core

F32 = mybir.dt.float32
F32R = mybir.dt.float32r
BF16 = mybir.dt.bfloat16
AX = mybir.AxisListType.X
OP = mybir.AluOpType
ACTF = mybir.ActivationFunctionType


def build_program(s=S, nph=4, qb=QB, n_warm=9):
    kp = s // nph             # k rows per phase
    kc = kp // P              # 128-chunks of k per phase
    nqt = qb // P             # q tiles per core
    dc = D // P               # contraction chunks
    net = D // 512            # AV output tiles
    nh = 2                    # k-halves per phase (single-DMA HT tiles)
    hw = kp // nh             # 512 keys per half

    nc = bacc.Bacc("TRN2", target_bir_lowering=False, debug=False)
    # ht arrives host-preswizzled: ht[ph, h, p, c, k] = h.T[c*128+p,
    # ph*kp + h*hw + k], so each phase-half is one contiguous block
    # (128 partitions x 16KB descriptors).
    ht = nc.dram_tensor("ht", [nph, 2, P, D // P, kp // 2], F32R,
                        kind="ExternalInput")
    v = nc.dram_tensor("v", [s, D], BF16, kind="ExternalInput")
    # m arrives pre-swizzled as [r, p, c, j] = M[c*128+p, r*128+j] so chunk r
    # is one contiguous 512KB block (128 partitions x 4KB descriptors).
    m = nc.dram_tensor("m", [dc, P, dc, P], F32R, kind="ExternalInput")
    vvec = nc.dram_tensor("vvec", [D], F32, kind="ExternalInput")
    ebias = nc.dram_tensor("ebias", [P], F32, kind="ExternalInput")
    onesb = nc.dram_tensor("onesb", [P], BF16, kind="ExternalInput")
    o = nc.dram_tensor("o", [qb, D], F32, kind="ExternalOutput")

    with tile.TileContext(nc) as tc:
        with (
            tc.tile_pool(name="sb", bufs=1) as sb,
            tc.tile_pool(name="ps", bufs=1, space="PSUM") as ps,
        ):
            # ---- HAM warmup comes FIRST (one cheap memset on the GpSimd
            # queue, then the SWDGE load dispatches): f32 dummy matmuls
            # (4 cyc/row, ~0.9-1.7us each) keep the PE busy and the HAM
            # clock warm across the prologue DMA window.
            wsrc = sb.tile([P, 512], F32, tag="wsrc")
            nc.gpsimd.memset(wsrc[:], 0.0)
            for i in range(n_warm):
                pw = ps.tile([P, 512], F32, tag="pss", bufs=4, name=f"warm{i}")
                nc.tensor.matmul(pw[:], wsrc[:, 0:P], wsrc[:], start=True,
                                 stop=True)

            v_sb = sb.tile([P, dc], F32, tag="vvec")
            nc.sync.dma_start(
                v_sb[:], vvec.ap().rearrange("(c p) -> p c", p=P))
            eb_sb = sb.tile([P, 1], F32, tag="ebias")
            nc.sync.dma_start(
                eb_sb[:], ebias.ap().rearrange("(p c) -> p c", c=1))
            on_sb = sb.tile([P, 1], BF16, tag="onesb")
            nc.sync.dma_start(
                on_sb[:], onesb.ap().rearrange("(p c) -> p c", c=1))



            def load_v(ph):
                vpr = []
                for scn in range(kc):
                    r0 = ph * kp + scn * P
                    vr = sb.tile([P, D], BF16, tag="vpr", bufs=20,
                                 name=f"vr{ph}_{scn}")
                    nc.gpsimd.dma_start(vr[:], v.ap()[r0:r0 + P, :])
                    vpr.append(vr)
                return vpr

            # phase-0 loads: HT half 0 gates the first GT chains, M chunks
            # chase per-chain, HT half 1 gates only the n=1 chains (~14us of
            # PE work later), V(0) is not needed until AV(0) in phase 1.
            def load_ht_half(ph, h):
                t = sb.tile([P, dc, hw], F32R, tag="htp", bufs=2 * nh,
                            name=f"htn{ph}_{h}")
                nc.gpsimd.dma_start(t[:], ht.ap()[ph, h])
                return t

            htn = [load_ht_half(0, 0)]
            m_ch = []
            for r in range(dc):
                t = sb.tile([P, dc, P], F32R, tag="big", bufs=dc,
                            name=f"mch{r}")
                nc.gpsimd.dma_start(t[:], m.ap()[r])
                m_ch.append(t)
            htn.append(load_ht_half(0, 1))
            vpr = load_v(0)

            # ---- persistent state --------------------------------------
            out_q = [sb.tile([P, D], F32, tag="big", bufs=dc,
                             name=f"outq{qt}") for qt in range(nqt)]
            dens = sb.tile([P, nqt], F32, tag="dens")  # running denominator
            gt_sb = sb.tile([P, dc, qb], F32R, tag="gt")

            # ---- GT (phase 0 holds this core's own q rows) --------------
            assert kp >= qb, "phase 0 must cover the q block"
            for n in range(nh):
                for r in range(dc):
                    pg = ps.tile([P, hw], F32, tag="pss", bufs=4)
                    for c in range(dc):
                        nc.tensor.matmul(
                            pg[:], m_ch[r][:, c, :], htn[n][:, c, :],
                            start=(c == 0), stop=(c == dc - 1),
                        )
                    # GT = psum + vvec[d] (ACT Identity bias folds it)
                    nc.scalar.activation(
                        gt_sb[:, r, n * hw:(n + 1) * hw], pg[:],
                        ACTF.Identity, bias=v_sb[:, r:r + 1], scale=1.0,
                    )

            # ---- phase machinery ---------------------------------------
            def av_drain(ph, qt, pav, pden, fin_tag=True):
                """DVE drain of an AV+den chain into the accumulators."""
                last_phase = ph == nph - 1
                d_run = dens[:, qt:qt + 1]
                if ph == 0:
                    nc.vector.tensor_copy(d_run, pden[:])
                else:
                    # ACT, not DVE: in the final-phase epilogue the DVE is
                    # nearly saturated by the normalize+store chain, and a
                    # queued den-accumulate delays the pden psum release
                    # (stalling the next q-tile's PE chain); ACT is idle.
                    nc.scalar.activation(d_run, pden[:], ACTF.Identity,
                                         bias=d_run, scale=1.0)
                if last_phase:
                    fin = sb.tile([P, 1], F32, tag="fin", bufs=3,
                                  name=f"fin{qt}")
                    nc.vector.reciprocal(fin[:], d_run)
                    nc.vector.tensor_scalar_mul(
                        out_q[qt][:], out_q[qt][:], fin[:])
                # 256-wide combine+store pieces on the last q-tile let the
                # final stores dispatch as soon as each quarter is ready
                sw_ = 256 if (last_phase and qt == nqt - 1) else 512
                for et in range(net):
                    if ph == 0:
                        nc.vector.tensor_copy(
                            out_q[qt][:, et * 512:(et + 1) * 512], pav[et][:])
                        continue
                    for j in range(512 // sw_):
                        o0 = et * 512 + j * sw_
                        dst = out_q[qt][:, o0:o0 + sw_]
                        src = pav[et][:, j * sw_:(j + 1) * sw_]
                        if not last_phase:
                            nc.vector.tensor_tensor(dst, dst, src, op=OP.add)
                        else:
                            # out = pav*fin + prescaled accumulator
                            nc.vector.scalar_tensor_tensor(
                                dst, src, fin[:], dst, op0=OP.mult,
                                op1=OP.add)
                            nc.sync.dma_start(
                                o.ap()[qt * P:(qt + 1) * P, o0:o0 + sw_], dst)

            def stage_s(ph, i, htn, eps):
                """Transposed score chunk i: st[k=128, q=1024] -> exp."""
                h, off = i // (kc // nh), (i % (kc // nh)) * P
                pst = [ps.tile([P, 512], F32, tag="pss", bufs=4,
                               name=f"pst{ph}_{i}_{n}") for n in range(2)]
                for c in range(dc):
                    for n in range(2):
                        nc.tensor.matmul(
                            pst[n][:], htn[h][:, c, off:off + P],
                            gt_sb[:, c, n * 512:(n + 1) * 512],
                            start=(c == 0), stop=(c == dc - 1),
                        )
                ep = sb.tile([P, qb], BF16, tag="ep", bufs=16,
                             name=f"ep{ph}_{i}")
                for n in range(2):
                    nc.scalar.activation(
                        ep[:, n * 512:(n + 1) * 512], pst[n][:], ACTF.Exp,
                        bias=eb_sb[:], scale=1.0,
                    )
                eps.append(ep)

            def stage_avp(ph, qt, eps, vpr):
                """AV + denominator chains for q-tile qt of phase ph
                (phase-body variant, runs between score chunks)."""
                pav = [ps.tile([P, 512], F32, tag="pav", bufs=2,
                               name=f"pav{ph}_{qt}_{e}") for e in range(net)]
                pden = ps.tile([P, 1], F32, tag="pden", bufs=2,
                               name=f"pden{ph}_{qt}")
                q0 = qt * P
                for c in range(kc):
                    st = eps[c][:, q0:q0 + P]
                    # den first: the ap=1 matmul absorbs the LDWEIGHTS wait
                    # of this chunk's stationary instead of interrupting the
                    # two streaming AV matmuls mid-pipeline.
                    nc.tensor.matmul(pden[:], st, on_sb[:],
                                     start=(c == 0), stop=(c == kc - 1))
                    for et in range(net):
                        nc.tensor.matmul(
                            pav[et][:], st, vpr[c][:, et * 512:(et + 1) * 512],
                            start=(c == 0), stop=(c == kc - 1))
                av_drain(ph, qt, pav, pden)

            def stage_av(ph, qt, eps, vpr):
                """Standalone AV+den chains (final phase epilogue). The AV
                psums alternate between the pss and pav tags: the score psums
                are done, and 2 tags x 2 bufs breaks the back-to-back WAR
                serialization of consecutive q-tiles."""
                tail = qt == nqt - 1
                pav = [ps.tile([P, 512], F32, tag="pav" if qt % 2 else "pss",
                               bufs=2 if qt % 2 else 4,
                               name=f"fav{qt}_{i}") for i in range(net)]
                pden = ps.tile([P, 1], F32, tag="pden", bufs=2,
                               name=f"fden{qt}")
                q0 = qt * P
                for c in range(kc):
                    st = eps[c][:, q0:q0 + P]
                    nc.tensor.matmul(pden[:], st, on_sb[:],
                                     start=(c == 0), stop=(c == kc - 1))
                    if not tail:
                        for et in range(net):
                            nc.tensor.matmul(
                                pav[et][:], st,
                                vpr[c][:, et * 512:(et + 1) * 512],
                                start=(c == 0), stop=(c == kc - 1))
                if tail:
                    # et-serial AV: the et=0 store overlaps et=1's matmuls
                    for et in range(net):
                        for c in range(kc):
                            nc.tensor.matmul(
                                pav[et][:], eps[c][:, q0:q0 + P],
                                vpr[c][:, et * 512:(et + 1) * 512],
                                start=(c == 0), stop=(c == kc - 1))
                av_drain(ph, qt, pav, pden)

            # ---- phase loop: AV(ph-1) interleaves with scores(ph) at
            # chunk granularity (MM-level interleave thrashes the PE weight
            # double-buffer: f32r<->bf16 stationary swaps every 2 MMs) -----
            prev = None
            for ph in range(nph):
                eps = []
                for i in range(kc):
                    stage_s(ph, i, htn, eps)
                    if i == 0 and ph + 1 < nph:
                        nxt_htn = [load_ht_half(ph + 1, h) for h in range(nh)]
                    if i == 6 and ph + 1 < nph:
                        nxt_vpr = load_v(ph + 1)
                    if prev is not None:
                        stage_avp(prev[0], i, prev[1], prev[2])
                prev = (ph, eps, vpr)
                if ph + 1 < nph:
                    htn, vpr = nxt_htn, nxt_vpr
            for qt in range(nqt):
                stage_av(nph - 1, qt, prev[1], prev[2])
    nc.compile()
    return nc


_PROGRAM = None


def _get_program():
    global _PROGRAM
    if _PROGRAM is None:
        _PROGRAM = build_program()
    return _PROGRAM


def _calibrate_ebias(h, M, vvec):
    """Global exp bias per batch. Softmax is shift-invariant, so any c with
    every row's max exp-argument inside [-87, +82] is exact: above -87 the
    row's dominant ep term stays fp32/bf16-normal (den > 0), below +82 the
    fp32 AV accumulators stay finite. Row maxes are computed exactly for 512
    sample rows; c sits at the midpoint of the feasible window, leaving ~20+
    units of margin on each side for this input family (rowmax spread ~125
    << window width 169)."""
    cs = []
    for b in range(h.shape[0]):
        rows = h[b, ::8]                         # [512, D]
        gs = rows @ M + vvec[None, :]            # [512, D]
        sc = gs @ h[b].T                         # [512, S] exact sample rows
        rm = sc.max(axis=1)
        cs.append((float(rm.max()) + float(rm.min()) + 5.0) / 2.0)
    return cs


def kernel(hidden_states, value_states, Wq, bq, Wk, bk):
    """Full-input entry point. Shards across 8 NeuronCores internally."""
    import ml_dtypes

    hidden_states = np.ascontiguousarray(
        np.asarray(hidden_states, dtype=np.float32))
    value_states = np.asarray(value_states, dtype=np.float32)
    value_states = value_states.astype(ml_dtypes.bfloat16)
    Wq = np.asarray(Wq, dtype=np.float64)
    Wk = np.asarray(Wk, dtype=np.float64)
    bq = np.asarray(bq, dtype=np.float64)
    # offline weight folding (exact in f64, cast once)
    M = (Wq.T @ Wk).astype(np.float32)
    vvec = np.ascontiguousarray((Wk.T @ bq).astype(np.float32))
    cs = _calibrate_ebias(hidden_states, M, vvec)
    # pre-swizzle M so device chunk r = [P, dc, P] is contiguous:
    # m_t[r, p, c, j] = M[c*128+p, r*128+j]
    dc = D // 128
    M = np.ascontiguousarray(
        M.reshape(dc, 128, dc, 128).transpose(2, 1, 0, 3))
    ones_b = np.ones(P, dtype=ml_dtypes.bfloat16)

    nc = _get_program()
    in_maps = []
    nph, hw = 4, 512
    for c in range(NCORES):
        b, qb = c // (NCORES // B), c % (NCORES // B)
        r0 = qb * QB
        # rotate rows so this core's q-block comes first (k-order invariant)
        hrot = np.concatenate(
            [hidden_states[b, r0:], hidden_states[b, :r0]], axis=0)
        # preswizzle h.T into [ph, h, p, c, k] contiguous phase-half blocks:
        # ht[ph, h, p, c, k] = hrot[ph*1024 + h*512 + k, c*128 + p]
        htr = np.ascontiguousarray(
            hrot.reshape(nph, 2, hw, D // P, P).transpose(0, 1, 4, 3, 2))
        vrot = np.ascontiguousarray(np.concatenate(
            [value_states[b, r0:], value_states[b, :r0]], axis=0))
        eb = np.full(P, -cs[b], dtype=np.float32)
        in_maps.append({"ht": htr, "v": vrot, "m": M, "vvec": vvec,
                        "ebias": eb, "onesb": ones_b})

    globals()["_LAST_IN_MAPS"] = in_maps
    res = run_bass_kernel_spmd(nc, in_maps, core_ids=list(range(NCORES)))

    out = np.empty((B, S, 1, D), dtype=np.float32)
    for c in range(NCORES):
        b, qb = c // (NCORES // B), c % (NCORES // B)
        out[b, qb * QB:(qb + 1) * QB, 0, :] = res.results[c]["o"]
    return out
